# revision 34
# baseline (speedup 1.0000x reference)
"""Trainium2 Bass kernel for nn_SpaceTimeAtten (space-time attention block).

Contract: kernel(**inputs) takes FULL unsharded numpy inputs (see reference
setup_inputs) and returns the FULL (2, 512, 8, 28, 28) float32 output.

Sharding: 8 cores = 2 batches x 4 query-chunks (T_LOC=1664 t-columns each).

Math restructure vs the straightforward staging (all exact up to bf16
rounding of bounded-value operands):
  - energy[t,s] = (Wh x_t + bh),(Wg x_s + bg) = x_t^T M x_s + beta_s + (per-t
    terms that cancel in softmax), M = Wh^T Wg.  The device computes
    u = M^T x once (one conv) and streams raw x_loc as the moving operand:
    the Q-projection disappears.  beta_s is folded into the exp bias
    (host-computed exactly).
  - mask_energy = Wh (mask @ attention^T) + bh: the PV matmul consumes the
    host-pre-transposed RAW mask (bf16), and the result is projected by Wh
    locally (1664 cols instead of 6272): the full-range mask projection
    disappears.
  - The second-softmax denominators are summed on the host (per-core partial
    sums are tiny outputs); gamma/SE scaling happens in the host combine.
    The device ships A = pm * expz and B = BN-normalized wy; the only
    collective is the BN-stats AllReduce, launched before attention so it is
    fully hidden.
  - Per t-block epilogue: normalization reads PV PSUM directly, transposes,
    projects, exponentiates and DMAs outputs while the next block computes.
"""

import numpy as np

# ---- problem constants (hardcoded per contract) ----
N_B, C, T, H, W = 2, 512, 8, 28, 28
THW = T * H * W            # 6272
BN_EPS = 1e-5

CI = 4                     # channel 128-chunks
CO = 4
NST = 49                   # s-tiles of 128 (exact)
T_LOC = 1664               # local t per core (13 tiles of 128)
NTT = 13
BLOCKS = [(0, 4), (4, 4), (8, 4), (12, 1)]   # (t-tile start, n tiles)
R_EPS = 1e-30

_PROG_CACHE = {}


def _build_program(m1, m2, debug=False):
    import concourse.bass as bass
    import concourse.mybir as mybir
    import concourse.tile as tile
    from concourse import bacc

    N_B, C = 2, 512
    THW = 6272
    BN_EPS = 1e-5
    CI = CO = 4
    NST = 49
    T_LOC = 1664
    NTT = 13
    BLOCKS = [(0, 4), (4, 4), (8, 4), (12, 1)]
    R_EPS = 1e-30

    f32 = mybir.dt.float32
    f32r = mybir.dt.float32r
    bf16 = mybir.dt.bfloat16
    EXP = mybir.ActivationFunctionType.Exp
    SQRT = mybir.ActivationFunctionType.Sqrt
    AX = mybir.AxisListType.X
    MUL = mybir.AluOpType.mult
    ADD = mybir.AluOpType.add

    nc = bacc.Bacc("TRN2")

    x_full = nc.dram_tensor("x_full", [C, THW], bf16, kind="ExternalInput")
    maskt = nc.dram_tensor("maskt", [128, NST * C], bf16, kind="ExternalInput")
    x_loc = nc.dram_tensor("x_loc", [C, T_LOC], bf16, kind="ExternalInput")
    uwt = nc.dram_tensor("uwt", [C, C], bf16, kind="ExternalInput")
    wht = nc.dram_tensor("wht", [C, C], bf16, kind="ExternalInput")
    wmt = nc.dram_tensor("wmt", [C, C], bf16, kind="ExternalInput")
    wzt = nc.dram_tensor("wzt", [C, C], bf16, kind="ExternalInput")
    bh_in = nc.dram_tensor("bh_in", [128, CO], f32, kind="ExternalInput")
    bm_in = nc.dram_tensor("bm_in", [128, CO], f32, kind="ExternalInput")
    bz_in = nc.dram_tensor("bz_in", [128, CO], f32, kind="ExternalInput")
    ones_in = nc.dram_tensor("ones_in", [128, 1], bf16, kind="ExternalInput")
    tmaddp_in = nc.dram_tensor("tmaddp_in", [128, 16], f32, kind="ExternalInput")
    beta_in = nc.dram_tensor("beta_in", [128, NST], f32, kind="ExternalInput")

    out_a = nc.dram_tensor("out_a", [C, T_LOC], f32, kind="ExternalOutput")
    out_wy = nc.dram_tensor("out_wy", [C, T_LOC], bf16, kind="ExternalOutput")
    se_out = nc.dram_tensor("se_out", [128, 16], f32, kind="ExternalOutput")
    stats_out = nc.dram_tensor("stats_out", [128, 8], f32, kind="ExternalOutput")
    if debug:
        d_u = nc.dram_tensor("d_u", [C, THW], bf16, kind="ExternalOutput")
        d_z = nc.dram_tensor("d_z", [C, T_LOC], f32, kind="ExternalOutput")
        d_r = nc.dram_tensor("d_r", [128, 16], f32, kind="ExternalOutput")
        d_wy = nc.dram_tensor("d_wy", [C, T_LOC], f32, kind="ExternalOutput")

    def dview(dram):
        return dram.rearrange("(k p) s -> p k s", p=128)

    with tile.TileContext(nc) as tc:
        with (
            tc.tile_pool(name="const", bufs=1) as cpool,
            tc.tile_pool(name="ptile", bufs=6) as ptpool,
            tc.tile_pool(name="small", bufs=1) as spool,
        ):
            # ---- constants (gpsimd queue) ----
            ones_t = cpool.tile([128, 1], bf16, tag="ones")
            nc.gpsimd.dma_start(out=ones_t[:], in_=ones_in[:])
            bh_t = cpool.tile([128, CO], f32, tag="bh")
            bm_t = cpool.tile([128, CO], f32, tag="bm")
            bz_t = cpool.tile([128, CO], f32, tag="bz")
            for tl, dr in ((bh_t, bh_in), (bm_t, bm_in), (bz_t, bz_in)):
                nc.gpsimd.dma_start(out=tl[:], in_=dr[:])
            tmaddp = cpool.tile([128, 16], f32, tag="tmaddp")
            nc.gpsimd.dma_start(out=tmaddp[:], in_=tmaddp_in[:])
            beta_t = cpool.tile([128, NST], f32, tag="beta")
            nc.gpsimd.dma_start(out=beta_t[:], in_=beta_in[:])
            m2b = cpool.tile([128, 1], f32, tag="m2b")
            nc.vector.memset(m2b[:], -m2)
            identb = cpool.tile([128, 128], bf16, tag="identb")
            from concourse.masks import make_identity
            make_identity(nc, identb[:])

            FC = T_LOC // 4  # 416

            # ---- pools, in stack order (early-released pools on top) ----
            p_w = tc.alloc_tile_pool(name="w1", bufs=1)
            wt_h = p_w.tile([128, CI, C], bf16, tag="wh")
            p_xl = tc.alloc_tile_pool(name="xlp", bufs=1)
            xloc = p_xl.tile([128, CI, T_LOC], bf16, tag="xloc")
            p_u = tc.alloc_tile_pool(name="up", bufs=1)
            u_t = p_u.tile([128, CI, THW], bf16, tag="u")
            p_mall = tc.alloc_tile_pool(name="mallp", bufs=1)
            mall = p_mall.tile([128, NST, C], bf16, tag="mall")
            p_wzm = tc.alloc_tile_pool(name="wzm", bufs=1)
            wt_z = p_wzm.tile([128, CI, C], bf16, tag="wz")
            wt_m = p_wzm.tile([128, CI, C], bf16, tag="wm")
            p_scr = tc.alloc_tile_pool(name="scrp", bufs=2)
            p_wu = tc.alloc_tile_pool(name="wu", bufs=1)
            wt_u = p_wu.tile([128, CI, C], bf16, tag="wu")
            p_piece = tc.alloc_tile_pool(name="piecep", bufs=3)

            # DMAs in priority order: xloc leads (wy/pm convs run first
            # and cover the u-conv piece-stream ramp-up)
            nc.sync.dma_start(out=wt_z[:], in_=dview(wzt))
            FCq = T_LOC // 4
            for fcq in range(4):
                nc.sync.dma_start(
                    out=xloc[:, :, fcq * FCq:(fcq + 1) * FCq],
                    in_=dview(x_loc)[:, :, fcq * FCq:(fcq + 1) * FCq])
            for ci, eng in enumerate((nc.sync, nc.sync, nc.gpsimd,
                                      nc.gpsimd)):
                eng.dma_start(out=wt_u[:, ci, :], in_=dview(uwt)[:, ci, :])
            nc.gpsimd.dma_start(out=wt_m[:], in_=dview(wmt))

            # ---- wy conv (f32r) + BN partial sums; launch CC early ----
            ps_c = tc.alloc_tile_pool(name="psc", bufs=4, space="PSUM")
            p_pmwy = tc.alloc_tile_pool(name="pmwyp", bufs=1, side="right")
            wy_bf = p_pmwy.tile([128, CO, T_LOC], bf16, tag="wy")
            pm_bf = p_pmwy.tile([128, CO, T_LOC], bf16, tag="pm")
            stats = spool.tile([128, 8], f32, tag="stats")
            stat_c = spool.tile([128, 32], f32, tag="statc")
            for fc in range(4):
                for co in range(CO):
                    ps = ps_c.tile([128, 512], f32, tag="c")
                    for ci in range(CI):
                        nc.tensor.matmul(
                            ps[:, :FC],
                            wt_z[:, ci, co * 128:(co + 1) * 128],
                            xloc[:, ci, fc * FC:(fc + 1) * FC],
                            start=(ci == 0), stop=(ci == CI - 1))
                    IDENT = mybir.ActivationFunctionType.Identity
                    SQUARE = mybir.ActivationFunctionType.Square
                    nc.scalar.activation(
                        wy_bf[:, co, fc * FC:(fc + 1) * FC], ps[:, :FC],
                        IDENT, bias=bz_t[:, co:co + 1], scale=1.0,
                        accum_out=stat_c[:, 4 * co + fc:4 * co + fc + 1])
                    scr = p_scr.tile([128, FC], f32, tag="scr")
                    wslice = wy_bf[:, co, fc * FC:(fc + 1) * FC]
                    nc.vector.tensor_mul(scr[:], wslice, wslice)
                    nc.vector.reduce_sum(
                        stat_c[:, 16 + 4 * co + fc:16 + 4 * co + fc + 1],
                        scr[:], axis=AX)
            # fold the 4 fc-partials per co into stats
            for co in range(CO):
                nc.vector.reduce_sum(stats[:, co:co + 1],
                                     stat_c[:, 4 * co:4 * co + 4], axis=AX)
                nc.vector.reduce_sum(stats[:, 4 + co:5 + co],
                                     stat_c[:, 16 + 4 * co:16 + 4 * co + 4],
                                     axis=AX)
            nc.gpsimd.dma_start(out=stats_out[:], in_=stats[:])
            for co in range(CO):
                nc.gpsimd.dma_start(out=dview(out_wy)[:, co, :],
                                    in_=wy_bf[:, co, :])

            # ---- pm conv ----
            for co in range(CO):
                for fc in range(4):
                    ps = ps_c.tile([128, 512], f32, tag="c")
                    for ci in range(CI):
                        nc.tensor.matmul(
                            ps[:, :FC],
                            wt_m[:, ci, co * 128:(co + 1) * 128],
                            xloc[:, ci, fc * FC:(fc + 1) * FC],
                            start=(ci == 0), stop=(ci == CI - 1))
                    nc.scalar.activation(
                        pm_bf[:, co, fc * FC:(fc + 1) * FC], ps[:, :FC],
                        mybir.ActivationFunctionType.Identity,
                        bias=bm_t[:, co:co + 1], scale=1.0)

            # ---- u conv: u = (Wg^T Wh) x over full s-range ----
            pieces = [(0, 1), (1, 2), (3, 3), (6, 4)]
            o = 10
            while o < NST:
                w = min(8, NST - o)
                pieces.append((o, w))
                o += w
            for pi, (pt0, ptw) in enumerate(pieces):
                s_off = pt0 * 128
                pw = ptw * 128
                xp = p_piece.tile([128, CI, 1024], bf16, tag="piece",
                                  name="xp")
                pc_eng = nc.scalar if pi % 2 == 0 else nc.sync
                pc_eng.dma_start(
                    out=xp[:, :, :pw],
                    in_=dview(x_full)[:, :, s_off:s_off + pw])
                if pi == 2:
                    nc.scalar.dma_start(out=wt_h[:], in_=dview(wht))
                for sub in range(0, ptw * 128, 512):
                    w2 = min(512, pw - sub)
                    for co in range(CO):
                        ps = ps_c.tile([128, 512], f32, tag="c")
                        for ci in range(CI):
                            nc.tensor.matmul(
                                ps[:, :w2],
                                wt_u[:, ci, co * 128:(co + 1) * 128],
                                xp[:, ci, sub:sub + w2],
                                start=(ci == 0), stop=(ci == CI - 1))
                        if co % 2 == 0:
                            nc.vector.tensor_copy(
                                u_t[:, co, s_off + sub:s_off + sub + w2],
                                ps[:, :w2])
                        else:
                            nc.scalar.activation(
                                u_t[:, co, s_off + sub:s_off + sub + w2],
                                ps[:, :w2],
                                mybir.ActivationFunctionType.Copy)
            if debug:
                nc.sync.dma_start(out=dview(d_u), in_=u_t[:])
            # resident mask: stream all 49 tiles once, split across
            # queues; first chunk leads, xloc (needed later, for the wy
            # conv) follows it so the piece stream is never starved
            for mi in range(8):
                st0 = mi * 6 + min(mi, 1)
                nck = 7 if mi == 0 else 6
                m_eng = nc.sync if mi % 2 == 0 else nc.scalar
                m_eng.dma_start(
                    out=mall[:, st0:st0 + nck, :],
                    in_=maskt[:, st0 * C:(st0 + nck) * C].rearrange(
                        "p (k c) -> p k c", k=nck))

            ps_c.release()
            p_piece.release()
            p_wu.release()
            p_scr.release()
            p_wzm.release()

            # ---- attention sweep: per t-block, 49 s-tiles, fused epilogue ----
            p_me = tc.alloc_tile_pool(name="mep", bufs=2)
            p_z = tc.alloc_tile_pool(name="zp", bufs=1)
            p_ez = tc.alloc_tile_pool(name="ezp", bufs=1)
            p_out = tc.alloc_tile_pool(name="outp", bufs=2)
            se_loc = spool.tile([128, 16], f32, tag="seloc")
            rrec = spool.tile([128, 16], f32, tag="rrec")
            ps_att = tc.alloc_tile_pool(name="psa", bufs=1, space="PSUM")
            if debug:
                dz_t = spool.tile([128, CO, T_LOC], f32, tag="dz")

            def emit_epilogue(bi, t0, nt, tfree, ops, rt):
                # r reciprocals: one strided reduce over the per-(j, st)
                # columns, then normalize + pad-mask + transpose to [o, t]
                nc.vector.reduce_sum(rrec[:, t0:t0 + nt], rt[:, :nt, :],
                                     axis=AX)
                nc.vector.tensor_scalar_add(rrec[:, t0:t0 + nt],
                                            rrec[:, t0:t0 + nt], R_EPS)
                nc.vector.reciprocal(rrec[:, t0:t0 + nt], rrec[:, t0:t0 + nt])
                zb = p_z.tile([128, CO, 512], bf16, tag="z", name="zb")
                mes = []
                for j in range(nt):
                    tt = t0 + j
                    me = p_me.tile([128, 512], bf16, tag="me", bufs=4)
                    nc.vector.tensor_scalar(me[:], ops[j][:],
                                            rrec[:, tt:tt + 1],
                                            tmaddp[:, tt:tt + 1],
                                            op0=MUL, op1=ADD)
                    mes.append(me)
                # oi-major transposes; the post-projection accumulates per
                # o-chunk as soon as its transposes land, reusing the ops
                # PSUM banks (already drained by the me pass above)
                ez = p_ez.tile([128, CO, 512], bf16, tag="ez", name="ez")
                ps2s = None
                for oi in range(CI):
                    for j in range(nt):
                        tp = ps_att.tile([128, 512], bf16, tag="e", bufs=3,
                                         name="tp")
                        nc.tensor.transpose(tp[:, :128],
                                            mes[j][:, oi * 128:(oi + 1) * 128],
                                            identb[:])
                        nc.scalar.activation(
                            zb[:, oi, j * 128:(j + 1) * 128], tp[:, :128],
                            mybir.ActivationFunctionType.Copy)
                    if oi == 0:
                        ps2s = [ps_att.tile([128, 512], f32, tag=f"o{co}",
                                            name=f"ps2{co}")
                                for co in range(CO)]
                    for co in range(CO):
                        nc.tensor.matmul(
                            ps2s[co][:, :tfree],
                            wt_h[:, oi, co * 128:(co + 1) * 128],
                            zb[:, oi, :tfree],
                            start=(oi == 0), stop=(oi == CI - 1))
                for co in range(CO):
                    z2 = p_me.tile([128, 512], f32, tag="z2", name="z2")
                    nc.vector.tensor_scalar_add(z2[:, :tfree],
                                                ps2s[co][:, :tfree],
                                                bh_t[:, co:co + 1])
                    if debug:
                        nc.sync.dma_start(
                            out=dview(d_z)[:, co, t0 * 128:t0 * 128 + tfree],
                            in_=z2[:, :tfree])
                    nc.scalar.activation(ez[:, co, :tfree], z2[:, :tfree],
                                         EXP, bias=m2b[:], scale=1.0,
                                         accum_out=se_loc[:, 4 * bi + co:
                                                          4 * bi + co + 1])
                    ot = p_out.tile([128, 512], f32, tag="ot", name="ot")
                    nc.vector.tensor_mul(ot[:, :tfree], ez[:, co, :tfree],
                                         pm_bf[:, co,
                                               t0 * 128:t0 * 128 + tfree])
                    nc.sync.dma_start(
                        out=dview(out_a)[:, co, t0 * 128:t0 * 128 + tfree],
                        in_=ot[:, :tfree])

            prev = None
            for bi, (t0, nt) in enumerate(BLOCKS):
                tfree = nt * 128
                pts = {}

                def emit_energy(st):
                    eps_t = ps_att.tile([128, 512], f32, tag="e", bufs=3,
                                        name="eps")
                    for ci in range(CI):
                        nc.tensor.matmul(
                            eps_t[:, :tfree],
                            u_t[:, ci, st * 128:(st + 1) * 128],
                            xloc[:, ci, t0 * 128:t0 * 128 + tfree],
                            start=(ci == 0), stop=(ci == CI - 1))
                    pt = ptpool.tile([128, 512], bf16, tag="pt")
                    nc.scalar.activation(pt[:, :tfree], eps_t[:, :tfree],
                                         EXP, bias=beta_t[:, st:st + 1],
                                         scale=1.0)
                    pts[st] = pt

                # software-pipeline: energy for st+1 is emitted before the
                # PV of st, so the PE covers the exp latency of tile st with
                # the energy matmuls of tile st+1.
                emit_energy(0)
                # previous block's epilogue rides inside this sweep: its
                # PSUM drains overlap this block's energy matmuls instead of
                # serializing at the boundary
                if prev is not None:
                    emit_epilogue(*prev)
                ops = [ps_att.tile([128, 512], f32, tag=f"o{j}",
                                   name=f"o{j}") for j in range(nt)]
                rt = ps_att.tile([128, CO, NST], f32, tag="r", name="rt")
                for st in range(NST):
                    if st + 1 < NST:
                        emit_energy(st + 1)
                    pt = pts.pop(st)
                    for j in range(nt):
                        nc.tensor.matmul(
                            ops[j][:],
                            pt[:, j * 128:(j + 1) * 128],
                            mall[:, st, :],
                            start=(st == 0), stop=(st == NST - 1))
                        # r row-sums ride on the already-loaded stationary
                        # pt: a 1-row matmul against a ones vector.  Each
                        # (st, j) writes its own column (start=stop=True) so
                        # no PSUM accumulation groups interleave.
                        nc.tensor.matmul(
                            rt[:, j, st:st + 1],
                            pt[:, j * 128:(j + 1) * 128],
                            ones_t[:],
                            start=True, stop=True)
                prev = (bi, t0, nt, tfree, ops, rt)
            emit_epilogue(*prev)

            nc.sync.dma_start(out=se_out[:], in_=se_loc[:])
            if debug:
                nc.sync.dma_start(out=d_r[:], in_=rrec[:])
                nc.sync.dma_start(out=dview(d_z), in_=dz_t[:])
                dwy = spool.tile([128, CO, T_LOC], f32, tag="dwy")
                for co in range(CO):
                    nc.vector.tensor_copy(dwy[:, co, :], wy_bf[:, co, :])
                nc.sync.dma_start(out=dview(d_wy), in_=dwy[:])

            ps_att.release()
            p_out.release()
            p_ez.release()
            p_z.release()
            p_me.release()
            p_mall.release()
            p_u.release()
            p_xl.release()
            p_w.release()
            p_pmwy.release()

    nc.compile()
    return nc


def _prepare_maps(x, mask, Wh, bh, Wg, bg, Wm, bm, Wz, bz, bn_w, bn_b, m1):
    import ml_dtypes

    xf = np.ascontiguousarray(x.reshape(N_B, C, THW), dtype=np.float32)
    mf = np.ascontiguousarray(mask.reshape(N_B, C, THW), dtype=np.float32)

    def chunked_bias(b):
        return np.ascontiguousarray(b.reshape(CO, 128).T, dtype=np.float32)

    U = (Wg.T @ Wh).astype(np.float32)        # u = U x
    uwt = np.ascontiguousarray(U.T.astype(ml_dtypes.bfloat16))
    wht = np.ascontiguousarray(Wh.T.astype(ml_dtypes.bfloat16))
    wmt = np.ascontiguousarray(Wm.T.astype(ml_dtypes.bfloat16))
    wzt = np.ascontiguousarray(Wz.T.astype(ml_dtypes.bfloat16))
    ones_bf = np.ones((128, 1), dtype=ml_dtypes.bfloat16)
    wgtbh = Wg.T @ bh                          # beta_s = (Wg^T bh) . x_s

    in_maps = []
    for core in range(8):
        n, q = divmod(core, 4)
        t0 = T_LOC * q
        valid = int(np.clip(THW - t0, 0, T_LOC))
        x_locc = np.zeros((C, T_LOC), ml_dtypes.bfloat16)
        x_locc[:, :valid] = xf[n][:, t0:t0 + valid].astype(ml_dtypes.bfloat16)
        x_fullc = np.ascontiguousarray(xf[n].astype(ml_dtypes.bfloat16))
        masktc = np.ascontiguousarray(
            mf[n].T.reshape(NST, 128, C).transpose(1, 0, 2).reshape(
                128, NST * C).astype(ml_dtypes.bfloat16))
        beta = wgtbh @ xf[n]                   # [THW]
        beta_dev = np.ascontiguousarray(
            (beta - m1).reshape(NST, 128).T, dtype=np.float32)
        # per-partition additive mask in [t-within-tile, t-tile] layout
        tmaddp = np.zeros((128, 16), np.float32)
        tgrid = (np.arange(NTT)[None, :] * 128 + np.arange(128)[:, None])
        tmaddp[:, :NTT] = np.where(tgrid < valid, 0.0, -1e30)
        in_maps.append(dict(
            x_full=x_fullc, maskt=masktc, x_loc=x_locc,
            uwt=uwt, wht=wht, wmt=wmt, wzt=wzt,
            bh_in=chunked_bias(bh),
            bm_in=chunked_bias(bm), bz_in=chunked_bias(bz),
            ones_in=ones_bf, tmaddp_in=tmaddp, beta_in=beta_dev,
        ))
    return in_maps


def _estimate_shifts(xf, mf, Wh, bh, Wg, bg):
    # M1: safe global upper bound for the max of the *effective* logits
    # l[t,s] = x_t^T (Wh^T Wg) x_s + (Wg^T bh).x_s (per-t terms cancel in
    # softmax).  Sampled estimate plus margin; any constant shift cancels.
    ti = np.arange(0, THW, 41)
    si = np.arange(0, THW, 7)
    M = (Wh.T @ Wg).astype(np.float64)
    wgtbh = (Wg.T @ bh).astype(np.float64)
    m_s = -np.inf
    for n in range(N_B):
        Qs = M.T @ xf[n][:, ti].astype(np.float64)      # columns M^T x_t
        l = Qs.T @ xf[n][:, si].astype(np.float64)
        l = l + (wgtbh @ xf[n][:, si].astype(np.float64))[None, :]
        m_s = max(m_s, float(l.max()))
    m1 = m_s + 5.0
    # M2: |mask_energy| bound: me = Wh @ (convex combos of mask cols) + bh.
    whn = float(np.linalg.norm(Wh, axis=1).max())
    mcn = max(float(np.linalg.norm(mf[n], axis=0).max()) for n in range(N_B))
    m2 = whn * mcn + float(np.abs(bh).max()) + 1.0
    return m1, m2


def kernel(x, mask, Wh, bh, Wg, bg, Wm, bm, Wz, bz, bn_w, bn_b, gamma,
           _debug=False, _trace=False):
    from concourse.bass_utils import run_bass_kernel_spmd

    x = np.asarray(x, np.float32)
    mask = np.asarray(mask, np.float32)
    Wh = np.asarray(Wh, np.float32); bh = np.asarray(bh, np.float32)
    Wg = np.asarray(Wg, np.float32); bg = np.asarray(bg, np.float32)
    Wm = np.asarray(Wm, np.float32); bm = np.asarray(bm, np.float32)
    Wz = np.asarray(Wz, np.float32); bz = np.asarray(bz, np.float32)
    bn_w = np.asarray(bn_w, np.float32); bn_b = np.asarray(bn_b, np.float32)
    gammaf = float(np.asarray(gamma))

    xf = x.reshape(N_B, C, THW)
    mf = mask.reshape(N_B, C, THW)
    m1, m2 = _estimate_shifts(xf, mf, Wh, bh, Wg, bg)
    key = (round(m1, 1), round(m2, 1), bool(_debug))
    if key not in _PROG_CACHE:
        _PROG_CACHE[key] = _build_program(key[0], key[1], debug=_debug)
    nc = _PROG_CACHE[key]

    in_maps = _prepare_maps(x, mask, Wh, bh, Wg, bg, Wm, bm, Wz, bz,
                            bn_w, bn_b, key[0])
    res = run_bass_kernel_spmd(nc, in_maps, core_ids=list(range(8)),
                               trace=_trace)

    # host combine: SE[n,c] = sum over the batch's 4 cores (and 4 t-blocks);
    # BN statistics = sum over all 8 cores, pad-compensated (pad columns
    # carry wy == bz exactly).
    se = np.zeros((N_B, 128, CO), np.float64)
    ssum = np.zeros((128, CO), np.float64)
    ssq = np.zeros((128, CO), np.float64)
    for core in range(8):
        n = core // 4
        se_raw = res.results[core]["se_out"].astype(np.float64)  # [128,16]
        se[n] += se_raw.reshape(128, 4, CO).sum(axis=1)
        st = res.results[core]["stats_out"].astype(np.float64)   # [128,8]
        ssum += st[:, 0:CO]
        ssq += st[:, CO:2 * CO]
    kfac = np.zeros((N_B, C), np.float32)
    for n in range(N_B):
        kfac[n] = gammaf / (se[n].T.reshape(C) + 1e-300)
    n_pad = 8 * T_LOC - N_B * THW
    cntf = 1.0 / (N_B * THW)
    bzch = bz.astype(np.float64).reshape(CO, 128).T
    mu = ssum * cntf - bzch * (n_pad * cntf)
    ex2 = ssq * cntf - (bzch * bzch) * (n_pad * cntf)
    var = ex2 - mu * mu
    bnwc = bn_w.astype(np.float64).reshape(CO, 128).T
    bnbc = bn_b.astype(np.float64).reshape(CO, 128).T
    alpha = (bnwc / np.sqrt(var + BN_EPS)).T.reshape(C).astype(np.float32)
    betab = (bnbc - mu * bnwc / np.sqrt(var + BN_EPS)).T.reshape(C).astype(
        np.float32)

    out = np.empty((N_B, C, THW), np.float32)
    for core in range(8):
        n, q = divmod(core, 4)
        t0 = T_LOC * q
        valid = int(np.clip(THW - t0, 0, T_LOC))
        if valid > 0:
            a = res.results[core]["out_a"][:, :valid]
            wy = res.results[core]["out_wy"][:, :valid].astype(np.float32)
            out[n][:, t0:t0 + valid] = (
                wy * alpha[:, None] + betab[:, None] + a * kfac[n][:, None])
    out = out.reshape(N_B, C, T, H, W)
    if _debug or _trace:
        return out, res
    return out


# revision 35
# speedup vs baseline: 1.0081x; 1.0081x over previous
"""Trainium2 Bass kernel for nn_SpaceTimeAtten (space-time attention block).

Contract: kernel(**inputs) takes FULL unsharded numpy inputs (see reference
setup_inputs) and returns the FULL (2, 512, 8, 28, 28) float32 output.

Sharding: 8 cores = 2 batches x 4 query-chunks (T_LOC=1664 t-columns each).

Math restructure vs the straightforward staging (all exact up to bf16
rounding of bounded-value operands):
  - energy[t,s] = (Wh x_t + bh),(Wg x_s + bg) = x_t^T M x_s + beta_s + (per-t
    terms that cancel in softmax), M = Wh^T Wg.  The device computes
    u = M^T x once (one conv) and streams raw x_loc as the moving operand:
    the Q-projection disappears.  beta_s is folded into the exp bias
    (host-computed exactly).
  - mask_energy = Wh (mask @ attention^T) + bh: the PV matmul consumes the
    host-pre-transposed RAW mask (bf16), and the result is projected by Wh
    locally (1664 cols instead of 6272): the full-range mask projection
    disappears.
  - The second-softmax denominators are summed on the host (per-core partial
    sums are tiny outputs); gamma/SE scaling happens in the host combine.
    The device ships A = pm * expz and B = BN-normalized wy; the only
    collective is the BN-stats AllReduce, launched before attention so it is
    fully hidden.
  - Per t-block epilogue: normalization reads PV PSUM directly, transposes,
    projects, exponentiates and DMAs outputs while the next block computes.
"""

import numpy as np

# ---- problem constants (hardcoded per contract) ----
N_B, C, T, H, W = 2, 512, 8, 28, 28
THW = T * H * W            # 6272
BN_EPS = 1e-5

CI = 4                     # channel 128-chunks
CO = 4
NST = 49                   # s-tiles of 128 (exact)
T_LOC = 1664               # local t per core (13 tiles of 128)
NTT = 13
BLOCKS = [(0, 4), (4, 4), (8, 3), (11, 2)]   # (t-tile start, n tiles)
R_EPS = 1e-30

_PROG_CACHE = {}


def _build_program(m1, m2, debug=False):
    import concourse.bass as bass
    import concourse.mybir as mybir
    import concourse.tile as tile
    from concourse import bacc

    N_B, C = 2, 512
    THW = 6272
    BN_EPS = 1e-5
    CI = CO = 4
    NST = 49
    T_LOC = 1664
    NTT = 13
    BLOCKS = [(0, 4), (4, 4), (8, 3), (11, 2)]
    R_EPS = 1e-30

    f32 = mybir.dt.float32
    f32r = mybir.dt.float32r
    bf16 = mybir.dt.bfloat16
    EXP = mybir.ActivationFunctionType.Exp
    SQRT = mybir.ActivationFunctionType.Sqrt
    AX = mybir.AxisListType.X
    MUL = mybir.AluOpType.mult
    ADD = mybir.AluOpType.add

    nc = bacc.Bacc("TRN2")

    x_full = nc.dram_tensor("x_full", [C, THW], bf16, kind="ExternalInput")
    maskt = nc.dram_tensor("maskt", [128, NST * C], bf16, kind="ExternalInput")
    x_loc = nc.dram_tensor("x_loc", [C, T_LOC], bf16, kind="ExternalInput")
    uwt = nc.dram_tensor("uwt", [C, C], bf16, kind="ExternalInput")
    wht = nc.dram_tensor("wht", [C, C], bf16, kind="ExternalInput")
    wmt = nc.dram_tensor("wmt", [C, C], bf16, kind="ExternalInput")
    wzt = nc.dram_tensor("wzt", [C, C], bf16, kind="ExternalInput")
    bh_in = nc.dram_tensor("bh_in", [128, CO], f32, kind="ExternalInput")
    bm_in = nc.dram_tensor("bm_in", [128, CO], f32, kind="ExternalInput")
    bz_in = nc.dram_tensor("bz_in", [128, CO], f32, kind="ExternalInput")
    ones_in = nc.dram_tensor("ones_in", [128, 1], bf16, kind="ExternalInput")
    tmaddp_in = nc.dram_tensor("tmaddp_in", [128, 16], f32, kind="ExternalInput")
    beta_in = nc.dram_tensor("beta_in", [128, NST], f32, kind="ExternalInput")

    out_a = nc.dram_tensor("out_a", [C, T_LOC], f32, kind="ExternalOutput")
    out_wy = nc.dram_tensor("out_wy", [C, T_LOC], bf16, kind="ExternalOutput")
    se_out = nc.dram_tensor("se_out", [128, 16], f32, kind="ExternalOutput")
    stats_out = nc.dram_tensor("stats_out", [128, 8], f32, kind="ExternalOutput")
    if debug:
        d_u = nc.dram_tensor("d_u", [C, THW], bf16, kind="ExternalOutput")
        d_z = nc.dram_tensor("d_z", [C, T_LOC], f32, kind="ExternalOutput")
        d_r = nc.dram_tensor("d_r", [128, 16], f32, kind="ExternalOutput")
        d_wy = nc.dram_tensor("d_wy", [C, T_LOC], f32, kind="ExternalOutput")

    def dview(dram):
        return dram.rearrange("(k p) s -> p k s", p=128)

    with tile.TileContext(nc) as tc:
        with (
            tc.tile_pool(name="const", bufs=1) as cpool,
            tc.tile_pool(name="ptile", bufs=6) as ptpool,
            tc.tile_pool(name="small", bufs=1) as spool,
        ):
            # ---- constants (gpsimd queue) ----
            ones_t = cpool.tile([128, 1], bf16, tag="ones")
            nc.gpsimd.dma_start(out=ones_t[:], in_=ones_in[:])
            bh_t = cpool.tile([128, CO], f32, tag="bh")
            bm_t = cpool.tile([128, CO], f32, tag="bm")
            bz_t = cpool.tile([128, CO], f32, tag="bz")
            for tl, dr in ((bh_t, bh_in), (bm_t, bm_in), (bz_t, bz_in)):
                nc.gpsimd.dma_start(out=tl[:], in_=dr[:])
            tmaddp = cpool.tile([128, 16], f32, tag="tmaddp")
            nc.gpsimd.dma_start(out=tmaddp[:], in_=tmaddp_in[:])
            beta_t = cpool.tile([128, NST], f32, tag="beta")
            nc.gpsimd.dma_start(out=beta_t[:], in_=beta_in[:])
            m2b = cpool.tile([128, 1], f32, tag="m2b")
            nc.vector.memset(m2b[:], -m2)
            identb = cpool.tile([128, 128], bf16, tag="identb")
            from concourse.masks import make_identity
            make_identity(nc, identb[:])

            FC = T_LOC // 4  # 416

            # ---- pools, in stack order (early-released pools on top) ----
            p_w = tc.alloc_tile_pool(name="w1", bufs=1)
            wt_h = p_w.tile([128, CI, C], bf16, tag="wh")
            p_xl = tc.alloc_tile_pool(name="xlp", bufs=1)
            xloc = p_xl.tile([128, CI, T_LOC], bf16, tag="xloc")
            p_u = tc.alloc_tile_pool(name="up", bufs=1)
            u_t = p_u.tile([128, CI, THW], bf16, tag="u")
            p_mall = tc.alloc_tile_pool(name="mallp", bufs=1)
            mall = p_mall.tile([128, NST, C], bf16, tag="mall")
            p_wzm = tc.alloc_tile_pool(name="wzm", bufs=1)
            wt_z = p_wzm.tile([128, CI, C], bf16, tag="wz")
            wt_m = p_wzm.tile([128, CI, C], bf16, tag="wm")
            p_scr = tc.alloc_tile_pool(name="scrp", bufs=2)
            p_wu = tc.alloc_tile_pool(name="wu", bufs=1)
            wt_u = p_wu.tile([128, CI, C], bf16, tag="wu")
            p_piece = tc.alloc_tile_pool(name="piecep", bufs=3)

            # DMAs in priority order: xloc leads (wy/pm convs run first
            # and cover the u-conv piece-stream ramp-up)
            nc.sync.dma_start(out=wt_z[:], in_=dview(wzt))
            FCq = T_LOC // 4
            for fcq in range(4):
                nc.sync.dma_start(
                    out=xloc[:, :, fcq * FCq:(fcq + 1) * FCq],
                    in_=dview(x_loc)[:, :, fcq * FCq:(fcq + 1) * FCq])
            for ci, eng in enumerate((nc.sync, nc.sync, nc.gpsimd,
                                      nc.gpsimd)):
                eng.dma_start(out=wt_u[:, ci, :], in_=dview(uwt)[:, ci, :])
            nc.gpsimd.dma_start(out=wt_m[:], in_=dview(wmt))

            # ---- wy conv (f32r) + BN partial sums; launch CC early ----
            ps_c = tc.alloc_tile_pool(name="psc", bufs=4, space="PSUM")
            p_pmwy = tc.alloc_tile_pool(name="pmwyp", bufs=1, side="right")
            wy_bf = p_pmwy.tile([128, CO, T_LOC], bf16, tag="wy")
            pm_bf = p_pmwy.tile([128, CO, T_LOC], bf16, tag="pm")
            stats = spool.tile([128, 8], f32, tag="stats")
            stat_c = spool.tile([128, 32], f32, tag="statc")
            for fc in range(4):
                for co in range(CO):
                    ps = ps_c.tile([128, 512], f32, tag="c")
                    for ci in range(CI):
                        nc.tensor.matmul(
                            ps[:, :FC],
                            wt_z[:, ci, co * 128:(co + 1) * 128],
                            xloc[:, ci, fc * FC:(fc + 1) * FC],
                            start=(ci == 0), stop=(ci == CI - 1))
                    IDENT = mybir.ActivationFunctionType.Identity
                    SQUARE = mybir.ActivationFunctionType.Square
                    nc.scalar.activation(
                        wy_bf[:, co, fc * FC:(fc + 1) * FC], ps[:, :FC],
                        IDENT, bias=bz_t[:, co:co + 1], scale=1.0,
                        accum_out=stat_c[:, 4 * co + fc:4 * co + fc + 1])
                    scr = p_scr.tile([128, FC], f32, tag="scr")
                    wslice = wy_bf[:, co, fc * FC:(fc + 1) * FC]
                    nc.vector.tensor_mul(scr[:], wslice, wslice)
                    nc.vector.reduce_sum(
                        stat_c[:, 16 + 4 * co + fc:16 + 4 * co + fc + 1],
                        scr[:], axis=AX)
            # fold the 4 fc-partials per co into stats
            for co in range(CO):
                nc.vector.reduce_sum(stats[:, co:co + 1],
                                     stat_c[:, 4 * co:4 * co + 4], axis=AX)
                nc.vector.reduce_sum(stats[:, 4 + co:5 + co],
                                     stat_c[:, 16 + 4 * co:16 + 4 * co + 4],
                                     axis=AX)
            nc.gpsimd.dma_start(out=stats_out[:], in_=stats[:])
            for co in range(CO):
                nc.gpsimd.dma_start(out=dview(out_wy)[:, co, :],
                                    in_=wy_bf[:, co, :])

            # ---- pm conv ----
            for co in range(CO):
                for fc in range(4):
                    ps = ps_c.tile([128, 512], f32, tag="c")
                    for ci in range(CI):
                        nc.tensor.matmul(
                            ps[:, :FC],
                            wt_m[:, ci, co * 128:(co + 1) * 128],
                            xloc[:, ci, fc * FC:(fc + 1) * FC],
                            start=(ci == 0), stop=(ci == CI - 1))
                    nc.scalar.activation(
                        pm_bf[:, co, fc * FC:(fc + 1) * FC], ps[:, :FC],
                        mybir.ActivationFunctionType.Identity,
                        bias=bm_t[:, co:co + 1], scale=1.0)

            # ---- u conv: u = (Wg^T Wh) x over full s-range ----
            pieces = [(0, 1), (1, 2), (3, 3), (6, 4)]
            o = 10
            while o < NST:
                w = min(8, NST - o)
                pieces.append((o, w))
                o += w
            for pi, (pt0, ptw) in enumerate(pieces):
                s_off = pt0 * 128
                pw = ptw * 128
                xp = p_piece.tile([128, CI, 1024], bf16, tag="piece",
                                  name="xp")
                pc_eng = nc.scalar if pi % 2 == 0 else nc.sync
                pc_eng.dma_start(
                    out=xp[:, :, :pw],
                    in_=dview(x_full)[:, :, s_off:s_off + pw])
                if pi == 2:
                    nc.scalar.dma_start(out=wt_h[:], in_=dview(wht))
                for sub in range(0, ptw * 128, 512):
                    w2 = min(512, pw - sub)
                    for co in range(CO):
                        ps = ps_c.tile([128, 512], f32, tag="c")
                        for ci in range(CI):
                            nc.tensor.matmul(
                                ps[:, :w2],
                                wt_u[:, ci, co * 128:(co + 1) * 128],
                                xp[:, ci, sub:sub + w2],
                                start=(ci == 0), stop=(ci == CI - 1))
                        if co % 2 == 0:
                            nc.vector.tensor_copy(
                                u_t[:, co, s_off + sub:s_off + sub + w2],
                                ps[:, :w2])
                        else:
                            nc.scalar.activation(
                                u_t[:, co, s_off + sub:s_off + sub + w2],
                                ps[:, :w2],
                                mybir.ActivationFunctionType.Copy)
            if debug:
                nc.sync.dma_start(out=dview(d_u), in_=u_t[:])
            # resident mask: stream all 49 tiles once, split across
            # queues; first chunk leads, xloc (needed later, for the wy
            # conv) follows it so the piece stream is never starved
            for mi in range(8):
                st0 = mi * 6 + min(mi, 1)
                nck = 7 if mi == 0 else 6
                m_eng = nc.sync if mi % 2 == 0 else nc.scalar
                m_eng.dma_start(
                    out=mall[:, st0:st0 + nck, :],
                    in_=maskt[:, st0 * C:(st0 + nck) * C].rearrange(
                        "p (k c) -> p k c", k=nck))

            ps_c.release()
            p_piece.release()
            p_wu.release()
            p_scr.release()
            p_wzm.release()

            # ---- attention sweep: per t-block, 49 s-tiles, fused epilogue ----
            p_me = tc.alloc_tile_pool(name="mep", bufs=2)
            p_z = tc.alloc_tile_pool(name="zp", bufs=1)
            p_ez = tc.alloc_tile_pool(name="ezp", bufs=1)
            p_out = tc.alloc_tile_pool(name="outp", bufs=2)
            se_loc = spool.tile([128, 16], f32, tag="seloc")
            rrec = spool.tile([128, 16], f32, tag="rrec")
            ps_att = tc.alloc_tile_pool(name="psa", bufs=1, space="PSUM")
            if debug:
                dz_t = spool.tile([128, CO, T_LOC], f32, tag="dz")

            def emit_epilogue(bi, t0, nt, tfree, ops, rt):
                # r reciprocals: one strided reduce over the per-(j, st)
                # columns, then normalize + pad-mask + transpose to [o, t]
                nc.vector.reduce_sum(rrec[:, t0:t0 + nt], rt[:, :nt, :],
                                     axis=AX)
                nc.vector.tensor_scalar_add(rrec[:, t0:t0 + nt],
                                            rrec[:, t0:t0 + nt], R_EPS)
                nc.vector.reciprocal(rrec[:, t0:t0 + nt], rrec[:, t0:t0 + nt])
                zb = p_z.tile([128, CO, 512], bf16, tag="z", name="zb")
                mes = []
                for j in range(nt):
                    tt = t0 + j
                    me = p_me.tile([128, 512], bf16, tag="me", bufs=4)
                    nc.vector.tensor_scalar(me[:], ops[j][:],
                                            rrec[:, tt:tt + 1],
                                            tmaddp[:, tt:tt + 1],
                                            op0=MUL, op1=ADD)
                    mes.append(me)
                # oi-major transposes; the post-projection accumulates per
                # o-chunk as soon as its transposes land, reusing the ops
                # PSUM banks (already drained by the me pass above)
                ez = p_ez.tile([128, CO, 512], bf16, tag="ez", name="ez")
                ps2s = None
                for oi in range(CI):
                    for j in range(nt):
                        tp = ps_att.tile([128, 512], bf16, tag="e", bufs=3,
                                         name="tp")
                        nc.tensor.transpose(tp[:, :128],
                                            mes[j][:, oi * 128:(oi + 1) * 128],
                                            identb[:])
                        nc.scalar.activation(
                            zb[:, oi, j * 128:(j + 1) * 128], tp[:, :128],
                            mybir.ActivationFunctionType.Copy)
                    if oi == 0:
                        ps2s = [ps_att.tile([128, 512], f32, tag=f"o{co}",
                                            name=f"ps2{co}")
                                for co in range(CO)]
                    for co in range(CO):
                        nc.tensor.matmul(
                            ps2s[co][:, :tfree],
                            wt_h[:, oi, co * 128:(co + 1) * 128],
                            zb[:, oi, :tfree],
                            start=(oi == 0), stop=(oi == CI - 1))
                for co in range(CO):
                    z2 = p_me.tile([128, 512], f32, tag="z2", name="z2")
                    nc.vector.tensor_scalar_add(z2[:, :tfree],
                                                ps2s[co][:, :tfree],
                                                bh_t[:, co:co + 1])
                    if debug:
                        nc.sync.dma_start(
                            out=dview(d_z)[:, co, t0 * 128:t0 * 128 + tfree],
                            in_=z2[:, :tfree])
                    nc.scalar.activation(ez[:, co, :tfree], z2[:, :tfree],
                                         EXP, bias=m2b[:], scale=1.0,
                                         accum_out=se_loc[:, 4 * bi + co:
                                                          4 * bi + co + 1])
                    ot = p_out.tile([128, 512], f32, tag="ot", name="ot")
                    nc.vector.tensor_mul(ot[:, :tfree], ez[:, co, :tfree],
                                         pm_bf[:, co,
                                               t0 * 128:t0 * 128 + tfree])
                    nc.sync.dma_start(
                        out=dview(out_a)[:, co, t0 * 128:t0 * 128 + tfree],
                        in_=ot[:, :tfree])

            prev = None
            for bi, (t0, nt) in enumerate(BLOCKS):
                tfree = nt * 128
                pts = {}

                def emit_energy(st):
                    eps_t = ps_att.tile([128, 512], f32, tag="e", bufs=3,
                                        name="eps")
                    for ci in range(CI):
                        nc.tensor.matmul(
                            eps_t[:, :tfree],
                            u_t[:, ci, st * 128:(st + 1) * 128],
                            xloc[:, ci, t0 * 128:t0 * 128 + tfree],
                            start=(ci == 0), stop=(ci == CI - 1))
                    pt = ptpool.tile([128, 512], bf16, tag="pt")
                    nc.scalar.activation(pt[:, :tfree], eps_t[:, :tfree],
                                         EXP, bias=beta_t[:, st:st + 1],
                                         scale=1.0)
                    pts[st] = pt

                # software-pipeline: energy for st+1 is emitted before the
                # PV of st, so the PE covers the exp latency of tile st with
                # the energy matmuls of tile st+1.
                emit_energy(0)
                # previous block's epilogue rides inside this sweep: its
                # PSUM drains overlap this block's energy matmuls instead of
                # serializing at the boundary
                if prev is not None:
                    emit_epilogue(*prev)
                ops = [ps_att.tile([128, 512], f32, tag=f"o{j}",
                                   name=f"o{j}") for j in range(nt)]
                rt = ps_att.tile([128, CO, NST], f32, tag="r", name="rt")
                for st in range(NST):
                    if st + 1 < NST:
                        emit_energy(st + 1)
                    pt = pts.pop(st)
                    for j in range(nt):
                        nc.tensor.matmul(
                            ops[j][:],
                            pt[:, j * 128:(j + 1) * 128],
                            mall[:, st, :],
                            start=(st == 0), stop=(st == NST - 1))
                        # r row-sums ride on the already-loaded stationary
                        # pt: a 1-row matmul against a ones vector.  Each
                        # (st, j) writes its own column (start=stop=True) so
                        # no PSUM accumulation groups interleave.
                        nc.tensor.matmul(
                            rt[:, j, st:st + 1],
                            pt[:, j * 128:(j + 1) * 128],
                            ones_t[:],
                            start=True, stop=True)
                prev = (bi, t0, nt, tfree, ops, rt)
            emit_epilogue(*prev)

            nc.sync.dma_start(out=se_out[:], in_=se_loc[:])
            if debug:
                nc.sync.dma_start(out=d_r[:], in_=rrec[:])
                nc.sync.dma_start(out=dview(d_z), in_=dz_t[:])
                dwy = spool.tile([128, CO, T_LOC], f32, tag="dwy")
                for co in range(CO):
                    nc.vector.tensor_copy(dwy[:, co, :], wy_bf[:, co, :])
                nc.sync.dma_start(out=dview(d_wy), in_=dwy[:])

            ps_att.release()
            p_out.release()
            p_ez.release()
            p_z.release()
            p_me.release()
            p_mall.release()
            p_u.release()
            p_xl.release()
            p_w.release()
            p_pmwy.release()

    nc.compile()
    return nc


def _prepare_maps(x, mask, Wh, bh, Wg, bg, Wm, bm, Wz, bz, bn_w, bn_b, m1):
    import ml_dtypes

    xf = np.ascontiguousarray(x.reshape(N_B, C, THW), dtype=np.float32)
    mf = np.ascontiguousarray(mask.reshape(N_B, C, THW), dtype=np.float32)

    def chunked_bias(b):
        return np.ascontiguousarray(b.reshape(CO, 128).T, dtype=np.float32)

    U = (Wg.T @ Wh).astype(np.float32)        # u = U x
    uwt = np.ascontiguousarray(U.T.astype(ml_dtypes.bfloat16))
    wht = np.ascontiguousarray(Wh.T.astype(ml_dtypes.bfloat16))
    wmt = np.ascontiguousarray(Wm.T.astype(ml_dtypes.bfloat16))
    wzt = np.ascontiguousarray(Wz.T.astype(ml_dtypes.bfloat16))
    ones_bf = np.ones((128, 1), dtype=ml_dtypes.bfloat16)
    wgtbh = Wg.T @ bh                          # beta_s = (Wg^T bh) . x_s

    in_maps = []
    for core in range(8):
        n, q = divmod(core, 4)
        t0 = T_LOC * q
        valid = int(np.clip(THW - t0, 0, T_LOC))
        x_locc = np.zeros((C, T_LOC), ml_dtypes.bfloat16)
        x_locc[:, :valid] = xf[n][:, t0:t0 + valid].astype(ml_dtypes.bfloat16)
        x_fullc = np.ascontiguousarray(xf[n].astype(ml_dtypes.bfloat16))
        masktc = np.ascontiguousarray(
            mf[n].T.reshape(NST, 128, C).transpose(1, 0, 2).reshape(
                128, NST * C).astype(ml_dtypes.bfloat16))
        beta = wgtbh @ xf[n]                   # [THW]
        beta_dev = np.ascontiguousarray(
            (beta - m1).reshape(NST, 128).T, dtype=np.float32)
        # per-partition additive mask in [t-within-tile, t-tile] layout
        tmaddp = np.zeros((128, 16), np.float32)
        tgrid = (np.arange(NTT)[None, :] * 128 + np.arange(128)[:, None])
        tmaddp[:, :NTT] = np.where(tgrid < valid, 0.0, -1e30)
        in_maps.append(dict(
            x_full=x_fullc, maskt=masktc, x_loc=x_locc,
            uwt=uwt, wht=wht, wmt=wmt, wzt=wzt,
            bh_in=chunked_bias(bh),
            bm_in=chunked_bias(bm), bz_in=chunked_bias(bz),
            ones_in=ones_bf, tmaddp_in=tmaddp, beta_in=beta_dev,
        ))
    return in_maps


def _estimate_shifts(xf, mf, Wh, bh, Wg, bg):
    # M1: safe global upper bound for the max of the *effective* logits
    # l[t,s] = x_t^T (Wh^T Wg) x_s + (Wg^T bh).x_s (per-t terms cancel in
    # softmax).  Sampled estimate plus margin; any constant shift cancels.
    ti = np.arange(0, THW, 41)
    si = np.arange(0, THW, 7)
    M = (Wh.T @ Wg).astype(np.float64)
    wgtbh = (Wg.T @ bh).astype(np.float64)
    m_s = -np.inf
    for n in range(N_B):
        Qs = M.T @ xf[n][:, ti].astype(np.float64)      # columns M^T x_t
        l = Qs.T @ xf[n][:, si].astype(np.float64)
        l = l + (wgtbh @ xf[n][:, si].astype(np.float64))[None, :]
        m_s = max(m_s, float(l.max()))
    m1 = m_s + 5.0
    # M2: |mask_energy| bound: me = Wh @ (convex combos of mask cols) + bh.
    whn = float(np.linalg.norm(Wh, axis=1).max())
    mcn = max(float(np.linalg.norm(mf[n], axis=0).max()) for n in range(N_B))
    m2 = whn * mcn + float(np.abs(bh).max()) + 1.0
    return m1, m2


def kernel(x, mask, Wh, bh, Wg, bg, Wm, bm, Wz, bz, bn_w, bn_b, gamma,
           _debug=False, _trace=False):
    from concourse.bass_utils import run_bass_kernel_spmd

    x = np.asarray(x, np.float32)
    mask = np.asarray(mask, np.float32)
    Wh = np.asarray(Wh, np.float32); bh = np.asarray(bh, np.float32)
    Wg = np.asarray(Wg, np.float32); bg = np.asarray(bg, np.float32)
    Wm = np.asarray(Wm, np.float32); bm = np.asarray(bm, np.float32)
    Wz = np.asarray(Wz, np.float32); bz = np.asarray(bz, np.float32)
    bn_w = np.asarray(bn_w, np.float32); bn_b = np.asarray(bn_b, np.float32)
    gammaf = float(np.asarray(gamma))

    xf = x.reshape(N_B, C, THW)
    mf = mask.reshape(N_B, C, THW)
    m1, m2 = _estimate_shifts(xf, mf, Wh, bh, Wg, bg)
    key = (round(m1, 1), round(m2, 1), bool(_debug))
    if key not in _PROG_CACHE:
        _PROG_CACHE[key] = _build_program(key[0], key[1], debug=_debug)
    nc = _PROG_CACHE[key]

    in_maps = _prepare_maps(x, mask, Wh, bh, Wg, bg, Wm, bm, Wz, bz,
                            bn_w, bn_b, key[0])
    res = run_bass_kernel_spmd(nc, in_maps, core_ids=list(range(8)),
                               trace=_trace)

    # host combine: SE[n,c] = sum over the batch's 4 cores (and 4 t-blocks);
    # BN statistics = sum over all 8 cores, pad-compensated (pad columns
    # carry wy == bz exactly).
    se = np.zeros((N_B, 128, CO), np.float64)
    ssum = np.zeros((128, CO), np.float64)
    ssq = np.zeros((128, CO), np.float64)
    for core in range(8):
        n = core // 4
        se_raw = res.results[core]["se_out"].astype(np.float64)  # [128,16]
        se[n] += se_raw.reshape(128, 4, CO).sum(axis=1)
        st = res.results[core]["stats_out"].astype(np.float64)   # [128,8]
        ssum += st[:, 0:CO]
        ssq += st[:, CO:2 * CO]
    kfac = np.zeros((N_B, C), np.float32)
    for n in range(N_B):
        kfac[n] = gammaf / (se[n].T.reshape(C) + 1e-300)
    n_pad = 8 * T_LOC - N_B * THW
    cntf = 1.0 / (N_B * THW)
    bzch = bz.astype(np.float64).reshape(CO, 128).T
    mu = ssum * cntf - bzch * (n_pad * cntf)
    ex2 = ssq * cntf - (bzch * bzch) * (n_pad * cntf)
    var = ex2 - mu * mu
    bnwc = bn_w.astype(np.float64).reshape(CO, 128).T
    bnbc = bn_b.astype(np.float64).reshape(CO, 128).T
    alpha = (bnwc / np.sqrt(var + BN_EPS)).T.reshape(C).astype(np.float32)
    betab = (bnbc - mu * bnwc / np.sqrt(var + BN_EPS)).T.reshape(C).astype(
        np.float32)

    out = np.empty((N_B, C, THW), np.float32)
    for core in range(8):
        n, q = divmod(core, 4)
        t0 = T_LOC * q
        valid = int(np.clip(THW - t0, 0, T_LOC))
        if valid > 0:
            a = res.results[core]["out_a"][:, :valid]
            wy = res.results[core]["out_wy"][:, :valid].astype(np.float32)
            out[n][:, t0:t0 + valid] = (
                wy * alpha[:, None] + betab[:, None] + a * kfac[n][:, None])
    out = out.reshape(N_B, C, T, H, W)
    if _debug or _trace:
        return out, res
    return out


# revision 37
# speedup vs baseline: 1.0202x; 1.0120x over previous
"""Trainium2 Bass kernel for nn_SpaceTimeAtten (space-time attention block).

Contract: kernel(**inputs) takes FULL unsharded numpy inputs (see reference
setup_inputs) and returns the FULL (2, 512, 8, 28, 28) float32 output.

Sharding: 8 cores = 2 batches x 4 query-chunks (T_LOC=1664 t-columns each).

Math restructure vs the straightforward staging (all exact up to bf16
rounding of bounded-value operands):
  - energy[t,s] = (Wh x_t + bh),(Wg x_s + bg) = x_t^T M x_s + beta_s + (per-t
    terms that cancel in softmax), M = Wh^T Wg.  The device computes
    u = M^T x once (one conv) and streams raw x_loc as the moving operand:
    the Q-projection disappears.  beta_s is folded into the exp bias
    (host-computed exactly).
  - mask_energy = Wh (mask @ attention^T) + bh: the PV matmul consumes the
    host-pre-transposed RAW mask (bf16), and the result is projected by Wh
    locally (1664 cols instead of 6272): the full-range mask projection
    disappears.
  - The second-softmax denominators are summed on the host (per-core partial
    sums are tiny outputs); gamma/SE scaling happens in the host combine.
    The device ships A = pm * expz and B = BN-normalized wy; the only
    collective is the BN-stats AllReduce, launched before attention so it is
    fully hidden.
  - Per t-block epilogue: normalization reads PV PSUM directly, transposes,
    projects, exponentiates and DMAs outputs while the next block computes.
"""

import numpy as np

# ---- problem constants (hardcoded per contract) ----
N_B, C, T, H, W = 2, 512, 8, 28, 28
THW = T * H * W            # 6272
BN_EPS = 1e-5

CI = 4                     # channel 128-chunks
CO = 4
NST = 49                   # s-tiles of 128 (exact)
T_LOC = 1664               # local t per core (13 tiles of 128)
NTT = 13
BLOCKS = [(0, 4), (4, 4), (8, 3), (11, 2)]   # (t-tile start, n tiles)
R_EPS = 1e-30

_PROG_CACHE = {}


def _build_program(m1, m2, debug=False):
    import concourse.bass as bass
    import concourse.mybir as mybir
    import concourse.tile as tile
    from concourse import bacc

    N_B, C = 2, 512
    THW = 6272
    BN_EPS = 1e-5
    CI = CO = 4
    NST = 49
    T_LOC = 1664
    NTT = 13
    BLOCKS = [(0, 4), (4, 4), (8, 3), (11, 2)]
    R_EPS = 1e-30

    f32 = mybir.dt.float32
    f32r = mybir.dt.float32r
    bf16 = mybir.dt.bfloat16
    EXP = mybir.ActivationFunctionType.Exp
    SQRT = mybir.ActivationFunctionType.Sqrt
    AX = mybir.AxisListType.X
    MUL = mybir.AluOpType.mult
    ADD = mybir.AluOpType.add

    nc = bacc.Bacc("TRN2")

    x_full = nc.dram_tensor("x_full", [C, THW], bf16, kind="ExternalInput")
    maskt = nc.dram_tensor("maskt", [128, NST * C], bf16, kind="ExternalInput")
    x_loc = nc.dram_tensor("x_loc", [C, T_LOC], bf16, kind="ExternalInput")
    uwt = nc.dram_tensor("uwt", [C, C], bf16, kind="ExternalInput")
    wht = nc.dram_tensor("wht", [C, C], bf16, kind="ExternalInput")
    wmt = nc.dram_tensor("wmt", [C, C], bf16, kind="ExternalInput")
    wzt = nc.dram_tensor("wzt", [C, C], bf16, kind="ExternalInput")
    bh_in = nc.dram_tensor("bh_in", [128, CO], f32, kind="ExternalInput")
    bm_in = nc.dram_tensor("bm_in", [128, CO], f32, kind="ExternalInput")
    bz_in = nc.dram_tensor("bz_in", [128, CO], f32, kind="ExternalInput")
    ones_in = nc.dram_tensor("ones_in", [128, 1], bf16, kind="ExternalInput")
    tmaddp_in = nc.dram_tensor("tmaddp_in", [128, 16], f32, kind="ExternalInput")
    beta_in = nc.dram_tensor("beta_in", [128, NST], f32, kind="ExternalInput")

    out_a = nc.dram_tensor("out_a", [C, T_LOC], f32, kind="ExternalOutput")
    out_wy = nc.dram_tensor("out_wy", [C, T_LOC], bf16, kind="ExternalOutput")
    se_out = nc.dram_tensor("se_out", [128, 16], f32, kind="ExternalOutput")
    stats_out = nc.dram_tensor("stats_out", [128, 8], f32, kind="ExternalOutput")
    if debug:
        d_u = nc.dram_tensor("d_u", [C, THW], bf16, kind="ExternalOutput")
        d_z = nc.dram_tensor("d_z", [C, T_LOC], f32, kind="ExternalOutput")
        d_r = nc.dram_tensor("d_r", [128, 16], f32, kind="ExternalOutput")
        d_wy = nc.dram_tensor("d_wy", [C, T_LOC], f32, kind="ExternalOutput")

    def dview(dram):
        return dram.rearrange("(k p) s -> p k s", p=128)

    with tile.TileContext(nc) as tc:
        with (
            tc.tile_pool(name="const", bufs=1) as cpool,
            tc.tile_pool(name="ptile", bufs=6) as ptpool,
            tc.tile_pool(name="small", bufs=1) as spool,
        ):
            # ---- constants (gpsimd queue) ----
            ones_t = cpool.tile([128, 1], bf16, tag="ones")
            nc.gpsimd.dma_start(out=ones_t[:], in_=ones_in[:])
            bh_t = cpool.tile([128, CO], f32, tag="bh")
            bm_t = cpool.tile([128, CO], f32, tag="bm")
            bz_t = cpool.tile([128, CO], f32, tag="bz")
            for tl, dr in ((bh_t, bh_in), (bm_t, bm_in), (bz_t, bz_in)):
                nc.gpsimd.dma_start(out=tl[:], in_=dr[:])
            tmaddp = cpool.tile([128, 16], f32, tag="tmaddp")
            nc.gpsimd.dma_start(out=tmaddp[:], in_=tmaddp_in[:])
            beta_t = cpool.tile([128, NST], f32, tag="beta")
            nc.gpsimd.dma_start(out=beta_t[:], in_=beta_in[:])
            m2b = cpool.tile([128, 1], f32, tag="m2b")
            nc.vector.memset(m2b[:], -m2)
            identb = cpool.tile([128, 128], bf16, tag="identb")
            from concourse.masks import make_identity
            make_identity(nc, identb[:])

            FC = T_LOC // 4  # 416

            # ---- pools, in stack order (early-released pools on top) ----
            p_w = tc.alloc_tile_pool(name="w1", bufs=1)
            wt_h = p_w.tile([128, CI, C], bf16, tag="wh")
            p_xl = tc.alloc_tile_pool(name="xlp", bufs=1)
            xloc = p_xl.tile([128, CI, T_LOC], bf16, tag="xloc")
            p_u = tc.alloc_tile_pool(name="up", bufs=1)
            u_t = p_u.tile([128, CI, THW], bf16, tag="u")
            p_mall = tc.alloc_tile_pool(name="mallp", bufs=1)
            mall = p_mall.tile([128, NST, C], bf16, tag="mall")
            p_wzm = tc.alloc_tile_pool(name="wzm", bufs=1)
            wt_z = p_wzm.tile([128, CI, C], bf16, tag="wz")
            wt_m = p_wzm.tile([128, CI, C], bf16, tag="wm")
            p_scr = tc.alloc_tile_pool(name="scrp", bufs=2)
            p_wu = tc.alloc_tile_pool(name="wu", bufs=1)
            wt_u = p_wu.tile([128, CI, C], bf16, tag="wu")
            p_piece = tc.alloc_tile_pool(name="piecep", bufs=3)

            # DMAs in priority order: xloc leads (wy/pm convs run first
            # and cover the u-conv piece-stream ramp-up)
            FCq = T_LOC // 4
            for fcq in range(4):
                nc.sync.dma_start(
                    out=xloc[:, :, fcq * FCq:(fcq + 1) * FCq],
                    in_=dview(x_loc)[:, :, fcq * FCq:(fcq + 1) * FCq])
            for ci, eng in enumerate((nc.sync, nc.sync, nc.gpsimd,
                                      nc.gpsimd)):
                eng.dma_start(out=wt_u[:, ci, :], in_=dview(uwt)[:, ci, :])
            nc.scalar.dma_start(out=wt_z[:], in_=dview(wzt))
            nc.gpsimd.dma_start(out=wt_m[:], in_=dview(wmt))

            # ---- wy conv (f32r) + BN partial sums; launch CC early ----
            ps_c = tc.alloc_tile_pool(name="psc", bufs=4, space="PSUM")
            p_pmwy = tc.alloc_tile_pool(name="pmwyp", bufs=1, side="right")
            wy_bf = p_pmwy.tile([128, CO, T_LOC], bf16, tag="wy")
            pm_bf = p_pmwy.tile([128, CO, T_LOC], bf16, tag="pm")
            stats = spool.tile([128, 8], f32, tag="stats")
            stat_c = spool.tile([128, 32], f32, tag="statc")
            for fc in range(4):
                for co in range(CO):
                    ps = ps_c.tile([128, 512], f32, tag="c")
                    for ci in range(CI):
                        nc.tensor.matmul(
                            ps[:, :FC],
                            wt_z[:, ci, co * 128:(co + 1) * 128],
                            xloc[:, ci, fc * FC:(fc + 1) * FC],
                            start=(ci == 0), stop=(ci == CI - 1))
                    IDENT = mybir.ActivationFunctionType.Identity
                    SQUARE = mybir.ActivationFunctionType.Square
                    nc.scalar.activation(
                        wy_bf[:, co, fc * FC:(fc + 1) * FC], ps[:, :FC],
                        IDENT, bias=bz_t[:, co:co + 1], scale=1.0,
                        accum_out=stat_c[:, 4 * co + fc:4 * co + fc + 1])
                    scr = p_scr.tile([128, FC], f32, tag="scr")
                    wslice = wy_bf[:, co, fc * FC:(fc + 1) * FC]
                    nc.vector.tensor_mul(scr[:], wslice, wslice)
                    nc.vector.reduce_sum(
                        stat_c[:, 16 + 4 * co + fc:16 + 4 * co + fc + 1],
                        scr[:], axis=AX)
            # fold the 4 fc-partials per co into stats
            for co in range(CO):
                nc.vector.reduce_sum(stats[:, co:co + 1],
                                     stat_c[:, 4 * co:4 * co + 4], axis=AX)
                nc.vector.reduce_sum(stats[:, 4 + co:5 + co],
                                     stat_c[:, 16 + 4 * co:16 + 4 * co + 4],
                                     axis=AX)
            nc.gpsimd.dma_start(out=stats_out[:], in_=stats[:])
            for co in range(CO):
                nc.gpsimd.dma_start(out=dview(out_wy)[:, co, :],
                                    in_=wy_bf[:, co, :])

            # ---- pm conv ----
            for co in range(CO):
                for fc in range(4):
                    ps = ps_c.tile([128, 512], f32, tag="c")
                    for ci in range(CI):
                        nc.tensor.matmul(
                            ps[:, :FC],
                            wt_m[:, ci, co * 128:(co + 1) * 128],
                            xloc[:, ci, fc * FC:(fc + 1) * FC],
                            start=(ci == 0), stop=(ci == CI - 1))
                    nc.scalar.activation(
                        pm_bf[:, co, fc * FC:(fc + 1) * FC], ps[:, :FC],
                        mybir.ActivationFunctionType.Identity,
                        bias=bm_t[:, co:co + 1], scale=1.0)

            # ---- u conv: u = (Wg^T Wh) x over full s-range ----
            pieces = [(0, 1), (1, 2), (3, 3), (6, 4)]
            o = 10
            while o < NST:
                w = min(8, NST - o)
                pieces.append((o, w))
                o += w
            for pi, (pt0, ptw) in enumerate(pieces):
                s_off = pt0 * 128
                pw = ptw * 128
                xp = p_piece.tile([128, CI, 1024], bf16, tag="piece",
                                  name="xp")
                pc_eng = nc.scalar if pi % 2 == 0 else nc.sync
                pc_eng.dma_start(
                    out=xp[:, :, :pw],
                    in_=dview(x_full)[:, :, s_off:s_off + pw])
                if pi == 2:
                    nc.scalar.dma_start(out=wt_h[:], in_=dview(wht))
                for sub in range(0, ptw * 128, 512):
                    w2 = min(512, pw - sub)
                    for co in range(CO):
                        ps = ps_c.tile([128, 512], f32, tag="c")
                        for ci in range(CI):
                            nc.tensor.matmul(
                                ps[:, :w2],
                                wt_u[:, ci, co * 128:(co + 1) * 128],
                                xp[:, ci, sub:sub + w2],
                                start=(ci == 0), stop=(ci == CI - 1))
                        if co % 2 == 0:
                            nc.vector.tensor_copy(
                                u_t[:, co, s_off + sub:s_off + sub + w2],
                                ps[:, :w2])
                        else:
                            nc.scalar.activation(
                                u_t[:, co, s_off + sub:s_off + sub + w2],
                                ps[:, :w2],
                                mybir.ActivationFunctionType.Copy)
            if debug:
                nc.sync.dma_start(out=dview(d_u), in_=u_t[:])
            # resident mask: stream all 49 tiles once, split across
            # queues; first chunk leads, xloc (needed later, for the wy
            # conv) follows it so the piece stream is never starved
            for mi in range(8):
                st0 = mi * 6 + min(mi, 1)
                nck = 7 if mi == 0 else 6
                m_eng = nc.sync if mi % 2 == 0 else nc.scalar
                m_eng.dma_start(
                    out=mall[:, st0:st0 + nck, :],
                    in_=maskt[:, st0 * C:(st0 + nck) * C].rearrange(
                        "p (k c) -> p k c", k=nck))

            ps_c.release()
            p_piece.release()
            p_wu.release()
            p_scr.release()
            p_wzm.release()

            # ---- attention sweep: per t-block, 49 s-tiles, fused epilogue ----
            p_me = tc.alloc_tile_pool(name="mep", bufs=2)
            p_z = tc.alloc_tile_pool(name="zp", bufs=1)
            p_ez = tc.alloc_tile_pool(name="ezp", bufs=1)
            p_out = tc.alloc_tile_pool(name="outp", bufs=2)
            se_loc = spool.tile([128, 16], f32, tag="seloc")
            rrec = spool.tile([128, 16], f32, tag="rrec")
            ps_att = tc.alloc_tile_pool(name="psa", bufs=1, space="PSUM")
            if debug:
                dz_t = spool.tile([128, CO, T_LOC], f32, tag="dz")

            def emit_epilogue(bi, t0, nt, tfree, ops, rt):
                # r reciprocals: one strided reduce over the per-(j, st)
                # columns, then normalize + pad-mask + transpose to [o, t]
                nc.vector.reduce_sum(rrec[:, t0:t0 + nt], rt[:, :nt, :],
                                     axis=AX)
                nc.vector.tensor_scalar_add(rrec[:, t0:t0 + nt],
                                            rrec[:, t0:t0 + nt], R_EPS)
                nc.vector.reciprocal(rrec[:, t0:t0 + nt], rrec[:, t0:t0 + nt])
                zb = p_z.tile([128, CO, 512], bf16, tag="z", name="zb")
                mes = []
                for j in range(nt):
                    tt = t0 + j
                    me = p_me.tile([128, 512], bf16, tag="me", bufs=4)
                    nc.vector.tensor_scalar(me[:], ops[j][:],
                                            rrec[:, tt:tt + 1],
                                            tmaddp[:, tt:tt + 1],
                                            op0=MUL, op1=ADD)
                    mes.append(me)
                # oi-major transposes; the post-projection accumulates per
                # o-chunk as soon as its transposes land, reusing the ops
                # PSUM banks (already drained by the me pass above)
                ez = p_ez.tile([128, CO, 512], bf16, tag="ez", name="ez")
                ps2s = None
                for oi in range(CI):
                    for j in range(nt):
                        tp = ps_att.tile([128, 512], bf16, tag="e", bufs=3,
                                         name="tp")
                        nc.tensor.transpose(tp[:, :128],
                                            mes[j][:, oi * 128:(oi + 1) * 128],
                                            identb[:])
                        nc.scalar.activation(
                            zb[:, oi, j * 128:(j + 1) * 128], tp[:, :128],
                            mybir.ActivationFunctionType.Copy)
                    if oi == 0:
                        ps2s = [ps_att.tile([128, 512], f32, tag=f"o{co}",
                                            name=f"ps2{co}")
                                for co in range(CO)]
                    for co in range(CO):
                        nc.tensor.matmul(
                            ps2s[co][:, :tfree],
                            wt_h[:, oi, co * 128:(co + 1) * 128],
                            zb[:, oi, :tfree],
                            start=(oi == 0), stop=(oi == CI - 1))
                for co in range(CO):
                    z2 = p_me.tile([128, 512], f32, tag="z2", name="z2")
                    nc.vector.tensor_scalar_add(z2[:, :tfree],
                                                ps2s[co][:, :tfree],
                                                bh_t[:, co:co + 1])
                    if debug:
                        nc.sync.dma_start(
                            out=dview(d_z)[:, co, t0 * 128:t0 * 128 + tfree],
                            in_=z2[:, :tfree])
                    nc.scalar.activation(ez[:, co, :tfree], z2[:, :tfree],
                                         EXP, bias=m2b[:], scale=1.0,
                                         accum_out=se_loc[:, 4 * bi + co:
                                                          4 * bi + co + 1])
                    ot = p_out.tile([128, 512], f32, tag="ot", name="ot")
                    nc.vector.tensor_mul(ot[:, :tfree], ez[:, co, :tfree],
                                         pm_bf[:, co,
                                               t0 * 128:t0 * 128 + tfree])
                    nc.sync.dma_start(
                        out=dview(out_a)[:, co, t0 * 128:t0 * 128 + tfree],
                        in_=ot[:, :tfree])

            prev = None
            for bi, (t0, nt) in enumerate(BLOCKS):
                tfree = nt * 128
                pts = {}

                def emit_energy(st):
                    eps_t = ps_att.tile([128, 512], f32, tag="e", bufs=3,
                                        name="eps")
                    for ci in range(CI):
                        nc.tensor.matmul(
                            eps_t[:, :tfree],
                            u_t[:, ci, st * 128:(st + 1) * 128],
                            xloc[:, ci, t0 * 128:t0 * 128 + tfree],
                            start=(ci == 0), stop=(ci == CI - 1))
                    pt = ptpool.tile([128, 512], bf16, tag="pt")
                    nc.scalar.activation(pt[:, :tfree], eps_t[:, :tfree],
                                         EXP, bias=beta_t[:, st:st + 1],
                                         scale=1.0)
                    pts[st] = pt

                # software-pipeline: energy for st+1 is emitted before the
                # PV of st, so the PE covers the exp latency of tile st with
                # the energy matmuls of tile st+1.
                emit_energy(0)
                # previous block's epilogue rides inside this sweep: its
                # PSUM drains overlap this block's energy matmuls instead of
                # serializing at the boundary
                if prev is not None:
                    emit_epilogue(*prev)
                ops = [ps_att.tile([128, 512], f32, tag=f"o{j}",
                                   name=f"o{j}") for j in range(nt)]
                rt = ps_att.tile([128, CO, NST], f32, tag="r", name="rt")
                for st in range(NST):
                    if st + 1 < NST:
                        emit_energy(st + 1)
                    pt = pts.pop(st)
                    for j in range(nt):
                        nc.tensor.matmul(
                            ops[j][:],
                            pt[:, j * 128:(j + 1) * 128],
                            mall[:, st, :],
                            start=(st == 0), stop=(st == NST - 1))
                        # r row-sums ride on the already-loaded stationary
                        # pt: a 1-row matmul against a ones vector.  Each
                        # (st, j) writes its own column (start=stop=True) so
                        # no PSUM accumulation groups interleave.
                        nc.tensor.matmul(
                            rt[:, j, st:st + 1],
                            pt[:, j * 128:(j + 1) * 128],
                            ones_t[:],
                            start=True, stop=True)
                prev = (bi, t0, nt, tfree, ops, rt)
            emit_epilogue(*prev)

            nc.sync.dma_start(out=se_out[:], in_=se_loc[:])
            if debug:
                nc.sync.dma_start(out=d_r[:], in_=rrec[:])
                nc.sync.dma_start(out=dview(d_z), in_=dz_t[:])
                dwy = spool.tile([128, CO, T_LOC], f32, tag="dwy")
                for co in range(CO):
                    nc.vector.tensor_copy(dwy[:, co, :], wy_bf[:, co, :])
                nc.sync.dma_start(out=dview(d_wy), in_=dwy[:])

            ps_att.release()
            p_out.release()
            p_ez.release()
            p_z.release()
            p_me.release()
            p_mall.release()
            p_u.release()
            p_xl.release()
            p_w.release()
            p_pmwy.release()

    nc.compile()
    return nc


def _prepare_maps(x, mask, Wh, bh, Wg, bg, Wm, bm, Wz, bz, bn_w, bn_b, m1):
    import ml_dtypes

    xf = np.ascontiguousarray(x.reshape(N_B, C, THW), dtype=np.float32)
    mf = np.ascontiguousarray(mask.reshape(N_B, C, THW), dtype=np.float32)

    def chunked_bias(b):
        return np.ascontiguousarray(b.reshape(CO, 128).T, dtype=np.float32)

    U = (Wg.T @ Wh).astype(np.float32)        # u = U x
    uwt = np.ascontiguousarray(U.T.astype(ml_dtypes.bfloat16))
    wht = np.ascontiguousarray(Wh.T.astype(ml_dtypes.bfloat16))
    wmt = np.ascontiguousarray(Wm.T.astype(ml_dtypes.bfloat16))
    wzt = np.ascontiguousarray(Wz.T.astype(ml_dtypes.bfloat16))
    ones_bf = np.ones((128, 1), dtype=ml_dtypes.bfloat16)
    wgtbh = Wg.T @ bh                          # beta_s = (Wg^T bh) . x_s

    in_maps = []
    for core in range(8):
        n, q = divmod(core, 4)
        t0 = T_LOC * q
        valid = int(np.clip(THW - t0, 0, T_LOC))
        x_locc = np.zeros((C, T_LOC), ml_dtypes.bfloat16)
        x_locc[:, :valid] = xf[n][:, t0:t0 + valid].astype(ml_dtypes.bfloat16)
        x_fullc = np.ascontiguousarray(xf[n].astype(ml_dtypes.bfloat16))
        masktc = np.ascontiguousarray(
            mf[n].T.reshape(NST, 128, C).transpose(1, 0, 2).reshape(
                128, NST * C).astype(ml_dtypes.bfloat16))
        beta = wgtbh @ xf[n]                   # [THW]
        beta_dev = np.ascontiguousarray(
            (beta - m1).reshape(NST, 128).T, dtype=np.float32)
        # per-partition additive mask in [t-within-tile, t-tile] layout
        tmaddp = np.zeros((128, 16), np.float32)
        tgrid = (np.arange(NTT)[None, :] * 128 + np.arange(128)[:, None])
        tmaddp[:, :NTT] = np.where(tgrid < valid, 0.0, -1e30)
        in_maps.append(dict(
            x_full=x_fullc, maskt=masktc, x_loc=x_locc,
            uwt=uwt, wht=wht, wmt=wmt, wzt=wzt,
            bh_in=chunked_bias(bh),
            bm_in=chunked_bias(bm), bz_in=chunked_bias(bz),
            ones_in=ones_bf, tmaddp_in=tmaddp, beta_in=beta_dev,
        ))
    return in_maps


def _estimate_shifts(xf, mf, Wh, bh, Wg, bg):
    # M1: safe global upper bound for the max of the *effective* logits
    # l[t,s] = x_t^T (Wh^T Wg) x_s + (Wg^T bh).x_s (per-t terms cancel in
    # softmax).  Sampled estimate plus margin; any constant shift cancels.
    ti = np.arange(0, THW, 41)
    si = np.arange(0, THW, 7)
    M = (Wh.T @ Wg).astype(np.float64)
    wgtbh = (Wg.T @ bh).astype(np.float64)
    m_s = -np.inf
    for n in range(N_B):
        Qs = M.T @ xf[n][:, ti].astype(np.float64)      # columns M^T x_t
        l = Qs.T @ xf[n][:, si].astype(np.float64)
        l = l + (wgtbh @ xf[n][:, si].astype(np.float64))[None, :]
        m_s = max(m_s, float(l.max()))
    m1 = m_s + 5.0
    # M2: |mask_energy| bound: me = Wh @ (convex combos of mask cols) + bh.
    whn = float(np.linalg.norm(Wh, axis=1).max())
    mcn = max(float(np.linalg.norm(mf[n], axis=0).max()) for n in range(N_B))
    m2 = whn * mcn + float(np.abs(bh).max()) + 1.0
    return m1, m2


def kernel(x, mask, Wh, bh, Wg, bg, Wm, bm, Wz, bz, bn_w, bn_b, gamma,
           _debug=False, _trace=False):
    from concourse.bass_utils import run_bass_kernel_spmd

    x = np.asarray(x, np.float32)
    mask = np.asarray(mask, np.float32)
    Wh = np.asarray(Wh, np.float32); bh = np.asarray(bh, np.float32)
    Wg = np.asarray(Wg, np.float32); bg = np.asarray(bg, np.float32)
    Wm = np.asarray(Wm, np.float32); bm = np.asarray(bm, np.float32)
    Wz = np.asarray(Wz, np.float32); bz = np.asarray(bz, np.float32)
    bn_w = np.asarray(bn_w, np.float32); bn_b = np.asarray(bn_b, np.float32)
    gammaf = float(np.asarray(gamma))

    xf = x.reshape(N_B, C, THW)
    mf = mask.reshape(N_B, C, THW)
    m1, m2 = _estimate_shifts(xf, mf, Wh, bh, Wg, bg)
    key = (round(m1, 1), round(m2, 1), bool(_debug))
    if key not in _PROG_CACHE:
        _PROG_CACHE[key] = _build_program(key[0], key[1], debug=_debug)
    nc = _PROG_CACHE[key]

    in_maps = _prepare_maps(x, mask, Wh, bh, Wg, bg, Wm, bm, Wz, bz,
                            bn_w, bn_b, key[0])
    res = run_bass_kernel_spmd(nc, in_maps, core_ids=list(range(8)),
                               trace=_trace)

    # host combine: SE[n,c] = sum over the batch's 4 cores (and 4 t-blocks);
    # BN statistics = sum over all 8 cores, pad-compensated (pad columns
    # carry wy == bz exactly).
    se = np.zeros((N_B, 128, CO), np.float64)
    ssum = np.zeros((128, CO), np.float64)
    ssq = np.zeros((128, CO), np.float64)
    for core in range(8):
        n = core // 4
        se_raw = res.results[core]["se_out"].astype(np.float64)  # [128,16]
        se[n] += se_raw.reshape(128, 4, CO).sum(axis=1)
        st = res.results[core]["stats_out"].astype(np.float64)   # [128,8]
        ssum += st[:, 0:CO]
        ssq += st[:, CO:2 * CO]
    kfac = np.zeros((N_B, C), np.float32)
    for n in range(N_B):
        kfac[n] = gammaf / (se[n].T.reshape(C) + 1e-300)
    n_pad = 8 * T_LOC - N_B * THW
    cntf = 1.0 / (N_B * THW)
    bzch = bz.astype(np.float64).reshape(CO, 128).T
    mu = ssum * cntf - bzch * (n_pad * cntf)
    ex2 = ssq * cntf - (bzch * bzch) * (n_pad * cntf)
    var = ex2 - mu * mu
    bnwc = bn_w.astype(np.float64).reshape(CO, 128).T
    bnbc = bn_b.astype(np.float64).reshape(CO, 128).T
    alpha = (bnwc / np.sqrt(var + BN_EPS)).T.reshape(C).astype(np.float32)
    betab = (bnbc - mu * bnwc / np.sqrt(var + BN_EPS)).T.reshape(C).astype(
        np.float32)

    out = np.empty((N_B, C, THW), np.float32)
    for core in range(8):
        n, q = divmod(core, 4)
        t0 = T_LOC * q
        valid = int(np.clip(THW - t0, 0, T_LOC))
        if valid > 0:
            a = res.results[core]["out_a"][:, :valid]
            wy = res.results[core]["out_wy"][:, :valid].astype(np.float32)
            out[n][:, t0:t0 + valid] = (
                wy * alpha[:, None] + betab[:, None] + a * kfac[n][:, None])
    out = out.reshape(N_B, C, T, H, W)
    if _debug or _trace:
        return out, res
    return out
